# revision 2
# baseline (speedup 1.0000x reference)
"""Trainium2 Bass kernel for the CoAttention scoring layer.

reference:
    keys    = receiver @ w_k                      # [B, R, D]
    queries = attendant @ w_q                     # [B, A, D]
    e_act   = queries[:, None, :, :] + keys[:, :, None, :] + bias  # [B, R, A, D]
    out     = tanh(e_act) @ a                     # [B, R, A]

Sharding: data-parallel over B across 8 NeuronCores (8 batches per core),
params replicated.

Algorithm: tanh(x) on the relevant range (|x| <= ~7.4) is approximated by
an odd-harmonic sine series tanh(x) ~= sum_m c_m sin(m*w0*x) over
m in {1,3,5,7,9}.  The angle-addition identity
    sin(w(u+v)) = sin(wu)cos(wv) + cos(wu)sin(wv)
makes every term separable in (u, v) = (query + bias, key), so the
[B, R, A, D] elementwise tensor never materializes: the D-contraction with
a_d becomes 2M=10 accumulating 128x128x128 matmuls per batch on the PE.

Per-core schedule (BC=8 batches, 2 groups of 4):
    PE:  qT/kT = w^T @ x^T wide fp16 matmuls into PSUM [D, 4*128]
    ACT: s1 = Sin(w0*qT + w0*bias_d), c1 = Sin(w0*qT + w0*bias_d + pi/2)
         (same for kT with bias 0 / pi/2); Square(s1).  |args| <= 3.62,
         inside the hw sin table + its cubic boundary extension.
    DVE: Chebyshev recurrences generate harmonics 3,5,7,9 from (s1, c2x2):
         s3=(c2x2+1)*s1, c3=(c2x2-1)*c1, s5=c2x2*s3-s1, ... (fp16, all
         values bounded in [-1,1], no range reduction needed)
    GPSIMD: q-side tiles scaled by per-partition a_d*c_m
    PE:  out[R, A] += kside_m^T @ qside_m for the 10 (m, sin/cos) pairs
"""

import sys

if "/opt/trn_rl_repo" not in sys.path:
    sys.path.insert(0, "/opt/trn_rl_repo")

from contextlib import ExitStack

import numpy as np

import concourse.bacc as bacc
import concourse.tile as tile
from concourse import mybir
from concourse.bass_utils import run_bass_kernel_spmd

B, R, A, F = 64, 128, 128, 256
D = F // 2
NCORES = 8
BC = B // NCORES   # batches per core
GB = 4             # batches per group
NG = BC // GB      # groups
W = GB * 128       # group free width (512)

W0 = 0.3512
CS = (1.2113, 0.2781, 0.0902, 0.0282, 0.0122)  # c_m for m = 1,3,5,7,9
M = len(CS)

F32 = mybir.dt.float32
F16 = mybir.dt.float16
SIN = mybir.ActivationFunctionType.Sin
SQUARE = mybir.ActivationFunctionType.Square
MULT = mybir.AluOpType.mult
ADD = mybir.AluOpType.add
SUB = mybir.AluOpType.subtract

_CACHE = {}


def build_bass():
    nc = bacc.Bacc("TRN2", target_bir_lowering=False, debug=False)

    # rat16[g, p, j, b*128+i]: j in 0,1 = receiverT f-chunks, 2,3 = attendantT
    rat_d = nc.declare_dram_parameter("rat16", [NG, 128, 4, W], F16, isOutput=False)
    wqk_d = nc.declare_dram_parameter("wqk16", [128, 4, 128], F16, isOutput=False)
    # bvec cols: 0 = w0*bias, 1 = w0*bias + pi/2, 2 = 0, 3 = pi/2, 4.. = a*c_m
    bvec_d = nc.declare_dram_parameter("bvec", [128, 4 + M], F32, isOutput=False)
    out_d = nc.declare_dram_parameter("out", [BC, R, A], F32, isOutput=True)

    with tile.TileContext(nc) as tc, ExitStack() as ctx:
        const = ctx.enter_context(tc.tile_pool(name="const", bufs=1))
        inp = ctx.enter_context(tc.tile_pool(name="inp", bufs=1))
        qkp = ctx.enter_context(tc.tile_pool(name="qkp", bufs=2, space="PSUM"))
        scp = ctx.enter_context(tc.tile_pool(name="scp", bufs=2, space="PSUM"))
        trig = ctx.enter_context(tc.tile_pool(name="trig", bufs=2))
        outp = ctx.enter_context(tc.tile_pool(name="outp", bufs=2))

        wqk_sb = const.tile([128, 4, 128], F16, tag="wqk")
        nc.sync.dma_start(wqk_sb[:], wqk_d[:])
        bvec = const.tile([128, 4 + M], F32, tag="bvec")
        nc.sync.dma_start(bvec[:], bvec_d[:])
        rat_sb = inp.tile([128, NG, 4, W], F16, tag="rat")
        for g in range(NG):
            nc.sync.dma_start(rat_sb[:, g], rat_d[g])

        for g in range(NG):
            rg = rat_sb[:, g]

            # kT[d, b*128+r], qT[d, b*128+a] for the 4 batches of the group
            kT_ps = qkp.tile([D, W], F32, tag="kT")
            nc.tensor.matmul(kT_ps[:], wqk_sb[:, 0], rg[:, 0], start=True, stop=False)
            nc.tensor.matmul(kT_ps[:], wqk_sb[:, 1], rg[:, 1], start=False, stop=True)
            qT_ps = qkp.tile([D, W], F32, tag="qT")
            nc.tensor.matmul(qT_ps[:], wqk_sb[:, 2], rg[:, 2], start=True, stop=False)
            nc.tensor.matmul(qT_ps[:], wqk_sb[:, 3], rg[:, 3], start=False, stop=True)

            # base sin/cos on ACT straight out of PSUM (fp16 out)
            su = [None] * M
            cu = [None] * M
            sv = [None] * M
            cv = [None] * M
            su[0] = trig.tile([D, W], F16, tag="su0", name=f"su0_{g}")
            nc.scalar.activation(su[0][:], qT_ps[:], SIN, bias=bvec[:, 0:1], scale=W0)
            cu[0] = trig.tile([D, W], F16, tag="cu0", name=f"cu0_{g}")
            nc.scalar.activation(cu[0][:], qT_ps[:], SIN, bias=bvec[:, 1:2], scale=W0)
            sv[0] = trig.tile([D, W], F16, tag="sv0", name=f"sv0_{g}")
            nc.scalar.activation(sv[0][:], kT_ps[:], SIN, bias=bvec[:, 2:3], scale=W0)
            cv[0] = trig.tile([D, W], F16, tag="cv0", name=f"cv0_{g}")
            nc.scalar.activation(cv[0][:], kT_ps[:], SIN, bias=bvec[:, 3:4], scale=W0)

            # Chebyshev ladders on DVE (c2x2 = 2*cos(2*w0*x) = 2 - 4*sin^2)
            for side, (s, c) in (("u", (su, cu)), ("v", (sv, cv))):
                sq = trig.tile([D, W], F16, tag=f"sq{side}", name=f"sq{side}_{g}")
                nc.scalar.activation(sq[:], s[0][:], SQUARE)
                c2 = trig.tile([D, W], F16, tag=f"c2{side}", name=f"c2{side}_{g}")
                nc.vector.tensor_scalar(c2[:], sq[:], -4.0, 2.0, MULT, ADD)
                s[1] = trig.tile([D, W], F16, tag=f"s1{side}", name=f"s1{side}_{g}")
                nc.vector.scalar_tensor_tensor(s[1][:], c2[:], 1.0, s[0][:], ADD, MULT)
                c[1] = trig.tile([D, W], F16, tag=f"c1{side}", name=f"c1{side}_{g}")
                nc.vector.scalar_tensor_tensor(c[1][:], c2[:], 1.0, c[0][:], SUB, MULT)
                for i in (2, 3, 4):
                    for sc, nm in ((s, "s"), (c, "c")):
                        t = trig.tile([D, W], F16, tag=f"t{nm}{i}{side}",
                                      name=f"t{nm}{i}{side}_{g}")
                        nc.vector.tensor_mul(t[:], c2[:], sc[i - 1][:])
                        sc[i] = trig.tile([D, W], F16, tag=f"{nm}{i}{side}",
                                          name=f"{nm}{i}{side}_{g}")
                        nc.vector.tensor_sub(sc[i][:], t[:], sc[i - 2][:])

            # q-side tiles scaled by a_d * c_m on gpsimd
            ps = [None] * M
            pc = [None] * M
            for m in range(M):
                ps[m] = trig.tile([D, W], F16, tag=f"ps{m}", name=f"ps{m}_{g}")
                nc.gpsimd.tensor_scalar_mul(ps[m][:], su[m][:], bvec[:, 4 + m:5 + m])
                pc[m] = trig.tile([D, W], F16, tag=f"pc{m}", name=f"pc{m}_{g}")
                nc.gpsimd.tensor_scalar_mul(pc[m][:], cu[m][:], bvec[:, 4 + m:5 + m])

            # out[r, b*128+a] = sum_m cv_m^T ps_m + sv_m^T pc_m per batch
            sc_ps = scp.tile([R, W], F32, tag="sc")
            for b in range(GB):
                sl = slice(b * 128, (b + 1) * 128)
                for m in range(M):
                    nc.tensor.matmul(sc_ps[:, sl], cv[m][:, sl], ps[m][:, sl],
                                     start=(m == 0), stop=False)
                    nc.tensor.matmul(sc_ps[:, sl], sv[m][:, sl], pc[m][:, sl],
                                     start=False, stop=(m == M - 1))

            sc_sb = outp.tile([R, GB, 128], F32, tag="sc_sb")
            nc.vector.tensor_copy(sc_sb[:], sc_ps[:].rearrange("r (b a) -> r b a", b=GB))
            for b in range(GB):
                nc.sync.dma_start(out_d[g * GB + b], sc_sb[:, b])

    nc.finalize()
    return nc


def _get_nc():
    if "nc" not in _CACHE:
        _CACHE["nc"] = build_bass()
    return _CACHE["nc"]


def make_in_maps(inputs):
    receiver = np.asarray(inputs["receiver"], dtype=np.float32)
    attendant = np.asarray(inputs["attendant"], dtype=np.float32)
    w_q = np.asarray(inputs["w_q"], dtype=np.float32)
    w_k = np.asarray(inputs["w_k"], dtype=np.float32)
    bias = np.asarray(inputs["bias"], dtype=np.float32)
    a = np.asarray(inputs["a"], dtype=np.float32)

    wqk16 = np.empty((128, 4, 128), dtype=np.float16)
    for j in range(2):
        wqk16[:, j] = w_k[j * 128:(j + 1) * 128]
        wqk16[:, 2 + j] = w_q[j * 128:(j + 1) * 128]

    bvec = np.zeros((128, 4 + M), dtype=np.float32)
    bvec[:, 0] = W0 * bias
    bvec[:, 1] = W0 * bias + np.pi / 2
    bvec[:, 2] = 0.0
    bvec[:, 3] = np.pi / 2
    for m in range(M):
        bvec[:, 4 + m] = a * CS[m]

    recvT = receiver.astype(np.float16).transpose(0, 2, 1)   # [B, F, R]
    attT = attendant.astype(np.float16).transpose(0, 2, 1)   # [B, F, A]

    in_maps = []
    for c in range(NCORES):
        rat = np.empty((NG, 128, 4, W), dtype=np.float16)
        for g in range(NG):
            b0 = c * BC + g * GB
            rb = recvT[b0:b0 + GB]   # [GB, F, R]
            ab = attT[b0:b0 + GB]
            for j in range(2):
                rat[g, :, j] = rb[:, j * 128:(j + 1) * 128].transpose(1, 0, 2).reshape(128, W)
                rat[g, :, 2 + j] = ab[:, j * 128:(j + 1) * 128].transpose(1, 0, 2).reshape(128, W)
        in_maps.append({
            "rat16": np.ascontiguousarray(rat),
            "wqk16": wqk16,
            "bvec": bvec,
        })
    return in_maps


def run(inputs, **kwargs):
    nc = _get_nc()
    in_maps = make_in_maps(inputs)
    res = run_bass_kernel_spmd(nc, in_maps, list(range(NCORES)), **kwargs)
    out = np.concatenate([res.results[c]["out"] for c in range(NCORES)], axis=0)
    return out, res


def kernel(**inputs) -> np.ndarray:
    out, _ = run(inputs)
    return out
